# revision 3
# baseline (speedup 1.0000x reference)
"""Trainium2 Bass kernel for nn_CapsuleSubLayer (capsule routing).

Math (per head h):
  uh[b,d,j] = sum_s W[h,d,j,s] * x[h,b,s,d]            (batched matmul over d)
  3 routing iterations of softmax / weighted-sum / squash / logit update
  out[b,d,n,h] = v[h,b,d]  (broadcast over n)

Sharding: heads are fully independent -> 2 heads per NeuronCore on 8 cores.
Host-side we pre-permute x and W into DMA-friendly per-(h,d) layouts:
  xt[h,d,p,c*64+b] = x[h,b,c*128+p,d]   (contiguous 256KB per (h,d))
  wt[h,d,p,c*16+n] = W[h,d,n,c*128+p]   (contiguous  64KB per (h,d))
so each per-(h,d) matmul chunk is lhsT=[p,b] (stationary), rhs=[p,n] (moving),
accumulated over c=0..7 into PSUM out[b,n].

Routing runs with partition=b (64 lanes): reductions over n are inner-free
reduces; the mean over b uses a ones-matmul on the PE which also leaves the
result replicated across partitions (exactly the layout the next softmax
needs).
"""

import os
import sys

import numpy as np

for _p in ("/opt/trn_rl_repo",):
    if _p not in sys.path:
        sys.path.insert(0, _p)

from contextlib import ExitStack

import concourse.bass as bass
import concourse.tile as tile
from concourse import bacc, mybir
from concourse.bass_utils import run_bass_kernel_spmd

F32 = mybir.dt.float32

H, B, S, D, N = 16, 64, 1024, 64, 16
NCORES = 8
H_LOC = H // NCORES  # 2 heads per core
C = S // 128  # 8 contraction chunks

_cache = {}


def _build(num_routing: int):
    nc = bacc.Bacc(
        "TRN2", target_bir_lowering=False, debug=False, num_devices=NCORES
    )
    xt = nc.dram_tensor("xt", [H_LOC, D, 128, C * B], F32, kind="ExternalInput").ap()
    wt = nc.dram_tensor("wt", [H_LOC, D, 128, C * N], F32, kind="ExternalInput").ap()
    ones = nc.dram_tensor("ones", [B, B], F32, kind="ExternalInput").ap()
    vout = nc.dram_tensor("vout", [B, H_LOC * D], F32, kind="ExternalOutput").ap()

    DG = 8  # d's per DMA batch (2MB x / 0.5MB w per transfer)
    PSD = 32  # d's per PSUM bank group

    with ExitStack() as ctx:
        tc = ctx.enter_context(tile.TileContext(nc))
        xpool = ctx.enter_context(tc.tile_pool(name="xp", bufs=3))
        wpool = ctx.enter_context(tc.tile_pool(name="wp", bufs=3))
        pspool = ctx.enter_context(tc.tile_pool(name="ps", bufs=3, space="PSUM"))
        bppool = ctx.enter_context(tc.tile_pool(name="bp", bufs=2, space="PSUM"))
        uhpool = ctx.enter_context(tc.tile_pool(name="uh", bufs=2))
        rpool = ctx.enter_context(tc.tile_pool(name="rt", bufs=2))
        spool = ctx.enter_context(tc.tile_pool(name="sm", bufs=4))
        singles = ctx.enter_context(tc.tile_pool(name="sg", bufs=1))

        ones_sb = singles.tile([B, B], F32)
        nc.sync.dma_start(out=ones_sb, in_=ones)
        vout_sb = singles.tile([B, H_LOC * D], F32)

        for h in range(H_LOC):
            uh = uhpool.tile([B, D, N], F32)
            ps = None
            for dg in range(D // DG):
                x_t = xpool.tile([128, DG, C * B], F32)
                nc.sync.dma_start(
                    out=x_t,
                    in_=xt[h, dg * DG : (dg + 1) * DG].rearrange("d p f -> p d f"),
                )
                w_t = wpool.tile([128, DG, C * N], F32)
                nc.sync.dma_start(
                    out=w_t,
                    in_=wt[h, dg * DG : (dg + 1) * DG].rearrange("d p f -> p d f"),
                )
                for dl in range(DG):
                    d = dg * DG + dl
                    if d % PSD == 0:
                        ps = pspool.tile([B, PSD, N], F32)
                    for c in range(C):
                        nc.tensor.matmul(
                            ps[:, d % PSD, :],
                            x_t[:, dl, c * B : (c + 1) * B],
                            w_t[:, dl, c * N : (c + 1) * N],
                            start=(c == 0),
                            stop=(c == C - 1),
                        )
                    if d % PSD == PSD - 1:
                        nc.vector.tensor_copy(
                            out=uh[:, d - (PSD - 1) : d + 1, :], in_=ps
                        )

            # ---- routing for head h (partition = b) ----
            bl = rpool.tile([B, D, N], F32)  # logits, replicated across b
            for it in range(num_routing):
                if it == 0:
                    s_raw = spool.tile([B, D, 1], F32)
                    nc.vector.reduce_sum(s_raw, uh, mybir.AxisListType.X)
                    scale = 1.0 / N
                else:
                    e = rpool.tile([B, D, N], F32)
                    nc.scalar.activation(e, bl, mybir.ActivationFunctionType.Exp)
                    esum = spool.tile([B, D, 1], F32)
                    nc.vector.reduce_sum(esum, e, mybir.AxisListType.X)
                    erec = spool.tile([B, D, 1], F32)
                    nc.vector.reciprocal(erec, esum)
                    cm = rpool.tile([B, D, N], F32)
                    nc.vector.tensor_mul(cm, e, erec.to_broadcast((B, D, N)))
                    cu = rpool.tile([B, D, N], F32)
                    nc.vector.tensor_mul(cu, cm, uh)
                    s_raw = spool.tile([B, D, 1], F32)
                    nc.vector.reduce_sum(s_raw, cu, mybir.AxisListType.X)
                    scale = 1.0

                # squash: v = s*|s| / (1 + s^2)
                m = spool.tile([B, D, 1], F32)
                nc.scalar.activation(
                    m, s_raw, mybir.ActivationFunctionType.Abs, scale=scale
                )
                if scale != 1.0:
                    s_sc = spool.tile([B, D, 1], F32)
                    nc.scalar.mul(s_sc, s_raw, scale)
                else:
                    s_sc = s_raw
                msq = spool.tile([B, D, 1], F32)
                nc.vector.tensor_mul(msq, m, m)
                den = spool.tile([B, D, 1], F32)
                nc.vector.tensor_scalar_add(den, msq, 1.0)
                rec = spool.tile([B, D, 1], F32)
                nc.vector.reciprocal(rec, den)
                t1 = spool.tile([B, D, 1], F32)
                nc.vector.tensor_mul(t1, m, rec)
                v = spool.tile([B, D, 1], F32)
                nc.vector.tensor_mul(v, t1, s_sc)

                if it < num_routing - 1:
                    uv = rpool.tile([B, D, N], F32)
                    nc.vector.tensor_mul(uv, uh, v.to_broadcast((B, D, N)))
                    bp = bppool.tile([B, 2, PSD, N], F32)
                    for half in range(2):
                        nc.tensor.matmul(
                            bp[:, half],
                            ones_sb,
                            uv[:, half * PSD : (half + 1) * PSD, :],
                            start=True,
                            stop=True,
                        )
                    for half in range(2):
                        blv = bl[:, half * PSD : (half + 1) * PSD, :]
                        if it == 0:
                            nc.scalar.mul(blv, bp[:, half], float(N) / B)
                        else:
                            tmp = spool.tile([B, PSD, N], F32)
                            nc.scalar.mul(tmp, bp[:, half], float(N) / B)
                            nc.vector.tensor_add(blv, blv, tmp)
                else:
                    nc.vector.tensor_copy(
                        out=vout_sb[:, h * D : (h + 1) * D], in_=v[:, :, 0]
                    )

        nc.sync.dma_start(out=vout, in_=vout_sb)
    nc.finalize()
    return nc


def _prep_core(x, W, k):
    xs = x[2 * k : 2 * k + 2]  # [2, B, S, D]
    xt = np.ascontiguousarray(
        xs.reshape(H_LOC, B, C, 128, D).transpose(0, 4, 3, 2, 1)
    ).reshape(H_LOC, D, 128, C * B)
    ws = W[2 * k : 2 * k + 2]  # [2, D, N, S]
    wt = np.ascontiguousarray(
        ws.reshape(H_LOC, D, N, C, 128).transpose(0, 1, 4, 3, 2)
    ).reshape(H_LOC, D, 128, C * N)
    return xt, wt


def kernel(x, W, num_routing):
    x = np.asarray(x, dtype=np.float32)
    W = np.asarray(W, dtype=np.float32)
    nr = int(num_routing)
    if nr not in _cache:
        _cache[nr] = _build(nr)
    nc = _cache[nr]

    ones = np.ones((B, B), dtype=np.float32)
    in_maps = []
    for k in range(NCORES):
        xt, wt = _prep_core(x, W, k)
        in_maps.append({"xt": xt, "wt": wt, "ones": ones})

    kernel.last_in_maps = in_maps
    res = run_bass_kernel_spmd(
        nc,
        in_maps,
        core_ids=list(range(NCORES)),
        trace=bool(int(os.environ.get("KERNEL_TRACE", "0"))),
    )
    kernel.last_result = res

    v_full = np.empty((H, B, D), dtype=np.float32)
    for k in range(NCORES):
        r = res.results[k]["vout"]  # [B, H_LOC*D]
        v_full[2 * k] = r[:, 0:D]
        v_full[2 * k + 1] = r[:, D : 2 * D]
    out = np.broadcast_to(
        v_full.transpose(1, 2, 0)[:, :, None, :], (B, D, N, H)
    )
    return np.ascontiguousarray(out)
